# revision 51
# baseline (speedup 1.0000x reference)
"""Multi-head attention with RoPE - Trainium2 Bass/Tile kernel.

Problem (hardcoded): B=2, S=2048, D=1024, H=16 heads, d_k=64, causal,
RoPE (theta=10000) on Q/K, fp32 reference.

Sharding: 8 cores = 2 batches x 4 head-groups (tensor parallel over heads,
data parallel over batch). Each core: QKV projections for its 4 heads,
RoPE, causal attention, and its o_proj row-slice -> partial [S, D] output.
Host gather sums the 4 partials per batch (the row-parallel all-reduce).

Fully-fused single-pipeline schedule (PE is the pacer at ~117us busy;
TimelineSim 144.7us vs the 210.8us phase-serial fp32 baseline):
  - All SBUF-resident tensors are bf16 (DMA halved, DVE 2x tensor-tensor);
    PSUM stays fp32.  Measured rel err ~5e-3 (gate is 2e-2).
  - One flex PSUM pool ([P,2,CH] x 2bufs = 4 banks) serves score pairs,
    projection psum groups, and o_proj psum; 4 more banks hold the
    per-head PV accumulators.  Projection/o_proj groups are interleaved
    into the attention k-tile loop as work-queue pops, so the pure-PE
    projection work fills every ACT-paced gap and qc-boundary bubble.
  - Scores for a head pair land in one [P,2,CH] tile -> ONE exp on ACT
    (bf16 out, causally restricted) -> PV accumulate per head.  PVs are
    software-pipelined one k-tile behind scores.  The causal diagonal is
    fixed by an in-place tril multiply on the exp'd stripe (DVE, off the
    critical path), keeping PV matmuls wide.
  - Softmax denominator comes from 64 ones-columns prepended per head in
    the V tile (psum partitions 0..63, base-0 for the DVE reciprocal);
    reciprocal_approx_fast + normalize folds into the attnT write
    (recips and normalize muls interleaved per head pair on DVE).
  - RoPE: Q/K psum is copied to bf16 (ACT), rotated on DVE via a
    stream_shuffle + 3 tensor-tensor ops against host-precomputed
    cos/sin tables (head dims pre-permuted so the rotation partner is a
    fixed 16-partition shuffle).
  - A burst of dummy matmuls on scratch warms the PE p-state under the
    initial DMA; outputs stream to DRAM per s-tile as o_proj completes.
"""

import numpy as np
import ml_dtypes

import concourse.tile as tile
from concourse import bacc, mybir
from concourse.bass_utils import run_bass_kernel_spmd

F32 = mybir.dt.float32
BF16 = mybir.dt.bfloat16
EXP = mybir.ActivationFunctionType.Exp
COPY = mybir.ActivationFunctionType.Copy

B, S, D, H, DK = 2, 2048, 1024, 16, 64
P = 128
NCORES = 8
HPC = 4  # heads per core
GD = HPC * DK  # 256 head dims per core
NDT = D // P  # 8 d-tiles
CH = 512  # q/s chunk
NQC = S // CH  # 4 chunks
KPC = CH // P  # 4 k-tiles per chunk
THETA = 10000.0
SCALE = 1.0 / 8.0  # 1/sqrt(DK)
SHUF_MASK = [(i + 16) % 32 for i in range(32)]

_CACHE = {}


def _build_nc():
    nc = bacc.Bacc("TRN2", target_bir_lowering=False, debug=False)
    # x arrives pre-transposed bf16: [D, S] (host layout prep)
    x = nc.dram_tensor("x", [D, S], BF16, kind="ExternalInput").ap()
    wq = nc.dram_tensor("wq", [P, 2, NDT, P], BF16, kind="ExternalInput").ap()
    wk = nc.dram_tensor("wk", [P, 2, NDT, P], BF16, kind="ExternalInput").ap()
    wv = nc.dram_tensor("wv", [P, NDT, GD], BF16, kind="ExternalInput").ap()
    wo = nc.dram_tensor("wo", [P, 2, D], BF16, kind="ExternalInput").ap()
    csf = nc.dram_tensor("csf", [P, 2, S], BF16, kind="ExternalInput").ap()
    dmask = nc.dram_tensor("dmask", [P, P], BF16, kind="ExternalInput").ap()
    out = nc.dram_tensor("out", [S, D], BF16, kind="ExternalOutput").ap()

    with tile.TileContext(nc) as tc:
        with (
            tc.tile_pool(name="const", bufs=1) as cpool,
            tc.tile_pool(name="wp", bufs=1) as wp,
            tc.tile_pool(name="csp", bufs=1) as csp,
            tc.tile_pool(name="xtp", bufs=4) as xtp,
            tc.tile_pool(name="rotp", bufs=1) as rotp,
            tc.tile_pool(name="vtp", bufs=1) as vtp,
            tc.tile_pool(name="attp", bufs=1) as attp,
            tc.tile_pool(name="qrp", bufs=4) as qrp,
            tc.tile_pool(name="shp", bufs=4) as shp,
            tc.tile_pool(name="t12", bufs=8) as t12,
            tc.tile_pool(name="ptp", bufs=20) as ptp,
            tc.tile_pool(name="rdp", bufs=6) as rdp,
            tc.tile_pool(name="osp", bufs=8) as osp,
            tc.tile_pool(name="ssp", bufs=2, space="PSUM") as ssp,
            tc.tile_pool(name="pvp", bufs=4, space="PSUM") as pvp,
        ):
            # ---- persistent SBUF tiles ----
            dmT = cpool.tile([P, P], BF16, tag="dm")
            scrT = cpool.tile([P, CH], BF16, tag="scr")
            woT = wp.tile([P, 2, D], BF16, tag="wo")
            wqT = wp.tile([P, 2, NDT, P], BF16, tag="wq")
            wkT = wp.tile([P, 2, NDT, P], BF16, tag="wk")
            wvT = wp.tile([P, NDT, GD], BF16, tag="wv")
            csC = [csp.tile([P, 2, CH], BF16, tag=f"cs{c}", name=f"cs{c}")
                   for c in range(NQC)]
            rotq = [[rotp.tile([P, CH], BF16, tag=f"rq{c}_{it}", name=f"rq{c}_{it}")
                     for it in range(2)] for c in range(NQC)]
            rotk = [[rotp.tile([P, CH], BF16, tag=f"rk{c}_{it}", name=f"rk{c}_{it}")
                     for it in range(2)] for c in range(NQC)]
            # per head: DK ones cols then DK value cols (softmax denominator
            # rides psum partitions 0..63 of the PV matmul).
            vt = [vtp.tile([P, KPC, HPC, 2 * DK], BF16, tag=f"vt{c}", name=f"vt{c}")
                  for c in range(NQC)]
            attn = [[attp.tile([P, CH], BF16, tag=f"at{c}_{i}", name=f"at{c}_{i}")
                     for i in range(2)] for c in range(NQC)]

            nc.gpsimd.memset(scrT[:], 0.0)

            # ---- input DMAs (SP queue), ordered by first use ----
            xrc = x.rearrange("(dt p) s -> p dt s", p=P)
            xt = []
            nc.sync.dma_start(wqT[:, 0], wq[:, 0])
            xt.append(xtp.tile([P, NDT, CH], BF16, tag="xt", name="xt0"))
            nc.sync.dma_start(xt[0][:, :, 0:CH // 2],
                              xrc[:, :, 0:CH // 2])
            nc.sync.dma_start(wqT[:, 1], wq[:, 1])
            nc.sync.dma_start(wkT[:, 0], wk[:, 0])
            nc.sync.dma_start(wkT[:, 1], wk[:, 1])
            nc.sync.dma_start(xt[0][:, :, CH // 2:CH],
                              xrc[:, :, CH // 2:CH])
            nc.sync.dma_start(wvT[:], wv)
            nc.sync.dma_start(csC[0][:], csf[:, :, 0:CH])
            nc.sync.dma_start(dmT[:], dmask)
            nc.sync.dma_start(woT[:], wo)
            for c in range(1, NQC):
                qsl = slice(c * CH, (c + 1) * CH)
                xt.append(xtp.tile([P, NDT, CH], BF16, tag="xt", name=f"xt{c}"))
                nc.sync.dma_start(xt[c][:], xrc[:, :, qsl])
                nc.sync.dma_start(csC[c][:], csf[:, :, qsl])

            # ones columns for the denominator (Pool, no deps)
            for c in range(NQC):
                for h in range(HPC):
                    nc.gpsimd.memset(vt[c][:, :, h, 0:DK], 1.0)

            # warm the PE p-state under the initial DMA: dummy matmuls on
            # scratch (results discarded; first real matmul then runs at
            # full clock)
            wps = ssp.tile([P, 2, CH], F32, tag="ss", name="warm")
            for _ in range(4):
                nc.tensor.matmul(wps[:, 0, :], scrT[0:P, 0:P], scrT[:],
                                 start=True, stop=True)

            # ---- work-queue pieces (popped into the kt loop) ----
            # half-width (256-moving) psum groups: each holds its flex
            # banks ~1.3us (856ns matmuls + ~400ns copy), matching the
            # exp/o_grp consumers so the 2-buffer rotation never stalls
            qraw_live = {}

            def qk_group(c, tgt, it, hw):
                """projection for one (q/k, it) half; RoPE after 2nd half"""
                wT, rot = (wqT, rotq[c]) if tgt == 0 else (wkT, rotk[c])
                HC = CH // 2
                hsl = slice(hw * HC, (hw + 1) * HC)
                ps = ssp.tile([P, 2, CH], F32, tag="ss",
                              name=f"qk{c}_{tgt}_{it}_{hw}")
                for dt in range(NDT):
                    nc.tensor.matmul(
                        ps[:, 0, 0:HC],
                        wT[:, it, dt, :],
                        xt[c][:, dt, hsl],
                        start=(dt == 0),
                        stop=(dt == NDT - 1),
                    )
                if hw == 0:
                    qraw_live[(c, tgt, it)] = qrp.tile(
                        [P, CH], BF16, tag="qraw",
                        name=f"qraw{c}_{tgt}_{it}")
                qraw = qraw_live[(c, tgt, it)]
                nc.scalar.activation(qraw[:, hsl], ps[:, 0, 0:HC], COPY)
                if hw == 1:
                    del qraw_live[(c, tgt, it)]
                    t1 = t12.tile([P, CH], BF16, tag="t1")
                    nc.vector.tensor_mul(t1[:], qraw[:], csC[c][:, 0, :])
                    sh = shp.tile([P, CH], BF16, tag="sh")
                    nc.vector.stream_shuffle(sh[:], qraw[:], SHUF_MASK)
                    t2 = t12.tile([P, CH], BF16, tag="t2")
                    nc.vector.tensor_mul(t2[:], sh[:], csC[c][:, 1, :])
                    nc.vector.tensor_add(rot[it][:], t1[:], t2[:])

            def v_group(c, stl):
                """V projection for one s-tile of chunk c"""
                ps = ssp.tile([P, 2, CH], F32, tag="ss", name=f"v{c}_{stl}")
                for dt in range(NDT):
                    nc.tensor.matmul(
                        ps[:, 0, 0:GD],
                        xt[c][:, dt, stl * P:(stl + 1) * P],
                        wvT[:, dt, :],
                        start=(dt == 0),
                        stop=(dt == NDT - 1),
                    )
                nc.scalar.activation(
                    vt[c][:, stl, :, DK:2 * DK],
                    ps[:, 0, 0:GD].rearrange("p (h d) -> p h d", h=HPC),
                    COPY,
                )

            def proj_chunk_work(c):
                w = []
                for tgt in range(2):
                    for it in range(2):
                        for hw in range(2):
                            w.append(lambda c=c, t=tgt, i=it, g=hw:
                                     qk_group(c, t, i, g))
                for stl in range(KPC):
                    w.append(lambda c=c, s=stl: v_group(c, s))
                return w

            def o_grp(qc, stl):
                """full-width o_proj for one s-tile (flex psum insertion)"""
                st = qc * KPC + stl
                psf = ssp.tile([P, 2, CH], F32, tag="ss", name=f"psf{st}")
                for jc in range(2):
                    jsl = slice(jc * CH, (jc + 1) * CH)
                    for itx in range(2):
                        # psum matmul dst must stay within one 2KB bank
                        nc.tensor.matmul(
                            psf[:, jc, :],
                            attn[qc][itx][:, stl * P:(stl + 1) * P],
                            woT[:, itx, jsl],
                            start=(itx == 0),
                            stop=(itx == 1),
                        )
                ost = osp.tile([P, D], BF16, tag="ost")
                # GPSIMD cannot read PSUM: one full-width DVE copy frees
                # the flex banks sooner than two serial half-copies
                nc.vector.tensor_copy(
                    ost[:], psf[:].rearrange('p a b -> p (a b)'))
                nc.sync.dma_start(out[st * P:(st + 1) * P, :], ost[:])

            # chunk-0 projections run up front, hw-major: the four
            # hw0 half-groups need only the first x column-half, so PE
            # rolls through all QK projections while x0b/wv stream in
            for hw in range(2):
                for tgt in range(2):
                    for it in range(2):
                        qk_group(0, tgt, it, hw)
            for stl in range(KPC):
                v_group(0, stl)

            work = []   # pending proj groups for the next chunk
            oq = []     # pending o_proj s-tiles
            # a few chunk-1 groups run before qc0 to fill the rope(0) wait
            head_start = proj_chunk_work(1)
            for w in head_start[:10]:
                w()
            prefetched = {1: 4

            # ---------------- fused attention pipeline ----------------
            for qc in range(NQC):
                nkt = (qc + 1) * KPC
                if qc < NQC - 1:
                    w_all = proj_chunk_work(qc + 1)
                    work.extend(w_all[prefetched.get(qc + 1, 0):])
                # spread proj pops across this qc's kts (all must land
                # before the qc boundary)
                pops_per_kt = -(-len(work) // nkt)

                psos = [pvp.tile([P, CH], F32, tag="pso", name=f"pso{qc}_{h}")
                        for h in range(HPC)]
                # two-deep PV pipeline: flush two kts behind so boundary
                # norm waits never gate the score chain
                pend = []

                def flush_front():
                    if not pend:
                        return
                    for dst, ptt, hf, vs, st_, sp_, lhs in pend.pop(0):
                        nc.tensor.matmul(dst, lhs, ptt[:, hf, vs:CH],
                                         start=st_, stop=sp_)

                for kt in range(nkt):
                    kc, ktl = kt // KPC, kt % KPC
                    dj = kt - KPC * qc
                    vs = max(0, dj) * P
                    # score width: restrict when it stays >=256 wide
                    svs = vs if dj > 0 else 0  # bf16 matmul has no narrow-width penalty
                    pts = []
                    for it in range(2):
                        ss = ssp.tile([P, 2, CH], F32, tag="ss",
                                      name=f"ss{qc}_{kt}_{it}")
                        for hf in range(2):
                            prg = slice(hf * DK, (hf + 1) * DK)
                            nc.tensor.matmul(
                                ss[:, hf, svs:CH],
                                rotk[kc][it][prg, ktl * P:(ktl + 1) * P],
                                rotq[qc][it][prg, svs:CH],
                                start=True, stop=True,
                            )
                        pt = ptp.tile([P, 2, CH], BF16, tag="pt")
                        nc.scalar.activation(
                            pt[:, :, vs:CH], ss[:, :, vs:CH], EXP,
                            scale=SCALE)
                        if dj >= 0:
                            # fix the diagonal stripe of both heads in
                            # place; the consuming PV fires next kt, so
                            # this DVE multiply is off the critical path
                            nc.vector.tensor_mul(
                                pt[:, :, vs:vs + P],
                                pt[:, :, vs:vs + P],
                                dmT[:].unsqueeze(1).to_broadcast([P, 2, P]),
                            )
                        pts.append(pt)
                    if len(pend) >= 1:
                        flush_front()
                    pend.append([
                        (psos[h][:, vs:CH], pts[h // 2], h % 2, vs,
                         kt == 0, kt == nkt - 1, vt[kc][:, ktl, h, :])
                        for h in range(HPC)])
                    # pops at kt END: the flex buffer grabbed next waits on
                    # a fast copy, not an exp; the popped group absorbs the
                    # exp latency instead of the score chain
                    for _ in range(pops_per_kt):
                        if work:
                            work.pop(0)()
                    if qc == NQC - 1 and kt >= 1 and oq:
                        oq.pop(0)()
                while work:
                    work.pop(0)()
                while pend:
                    flush_front()
                rdens = []
                for h in range(HPC):
                    rden = rdp.tile([DK, CH], F32, tag="rden",
                                    name=f"rden{qc}_{h}")
                    nc.vector.reciprocal_approx_fast(
                        out=rden[:], in_=psos[h][0:DK, :])
                    rdens.append(rden)
                    if h % 2 == 1:
                        for hh in (h - 1, h):
                            dst = attn[qc][hh // 2][
                                (hh % 2) * DK:(hh % 2 + 1) * DK, :]
                            nc.vector.tensor_mul(
                                dst, psos[hh][DK:P, :], rdens[hh][:])
                if qc < NQC - 1:
                    for stl in range(KPC):
                        oq.append(lambda q=qc, s=stl: o_grp(q, s))
                else:
                    # tail: the last qc's o_proj drains through the freed
                    # PV banks (deep rotation, no flex-pool coupling)
                    while oq:
                        oq.pop(0)()
                    for stl in range(KPC):
                        st = qc * KPC + stl
                        ost = osp.tile([P, D], BF16, tag="ost",
                                       name=f"ostt{st}")
                        for jc in range(2):
                            jsl = slice(jc * CH, (jc + 1) * CH)
                            psf = pvp.tile([P, CH], F32, tag="pso",
                                           name=f"psft{st}_{jc}")
                            for itx in range(2):
                                nc.tensor.matmul(
                                    psf[:],
                                    attn[qc][itx][:, stl * P:(stl + 1) * P],
                                    woT[:, itx, jsl],
                                    start=(itx == 0),
                                    stop=(itx == 1),
                                )
                            if jc == 0:
                                nc.vector.tensor_copy(ost[:, jsl], psf[:])
                            else:
                                nc.scalar.activation(ost[:, jsl], psf[:], COPY)
                        nc.sync.dma_start(out[st * P:(st + 1) * P, :], ost[:])
            while oq:
                oq.pop(0)()
    nc.compile()
    return nc


def _tables():
    r = np.arange(P)
    j = 16 * ((r % 64) // 32) + (r % 16)
    inv = THETA ** (-(2.0 * j) / DK)
    ang = np.arange(S)[None, :] * inv[:, None]
    cosf = np.cos(ang)
    sgn = np.where((r % 32) < 16, -1.0, 1.0)
    sinf = np.sin(ang) * sgn[:, None]
    csf = np.ascontiguousarray(
        np.stack([cosf, sinf], axis=1)).astype(ml_dtypes.bfloat16)
    dmask = np.where(
        np.arange(P)[:, None] <= np.arange(P)[None, :], 1.0, 0.0
    ).astype(ml_dtypes.bfloat16)  # tril01: 1 where k <= q
    return csf, dmask


def _head_perm():
    # sbuf row r (within a head) <- original head dim perm[r]:
    # windows of 32 rows = [16 even dims, 16 odd dims]
    r = np.arange(DK)
    w = r // 32
    idx = r % 32
    return np.where(idx < 16, 32 * w + 2 * idx, 32 * w + 2 * (idx - 16) + 1)


def _wqk_layout(w):  # [D, GD] -> [P, 2(it), NDT, P]
    return np.ascontiguousarray(
        w.reshape(NDT, P, 2, P).transpose(1, 2, 0, 3))


def _wv_layout(w):  # [D, GD] -> [P, NDT, GD]
    return np.ascontiguousarray(w.reshape(NDT, P, GD).transpose(1, 0, 2))


def _wo_layout(w):  # [GD, D] -> [P, 2(it), D]
    return np.ascontiguousarray(w.reshape(2, P, D).transpose(1, 0, 2))


LAST_RESULTS = None


def kernel(**inputs):
    global LAST_RESULTS
    bf = ml_dtypes.bfloat16
    x = np.asarray(inputs["in_features"], dtype=np.float32)
    qp = np.asarray(inputs["q_proj"], dtype=np.float32)
    kp = np.asarray(inputs["k_proj"], dtype=np.float32)
    vp = np.asarray(inputs["v_proj"], dtype=np.float32)
    op = np.asarray(inputs["o_proj"], dtype=np.float32)

    if "nc" not in _CACHE:
        _CACHE["nc"] = _build_nc()
        _CACHE["tables"] = _tables()
    nc = _CACHE["nc"]
    csf, dmask = _CACHE["tables"]
    perm = _head_perm()
    idx = (np.arange(HPC)[:, None] * DK + perm[None, :]).reshape(-1)

    in_maps = []
    for c in range(NCORES):
        b, g = c // 4, c % 4
        rows = slice(HPC * g * DK, HPC * (g + 1) * DK)
        in_maps.append(
            {
                "x": np.ascontiguousarray(x[b].T.astype(bf)),
                "wq": _wqk_layout(qp[rows, :][idx, :].T.astype(bf)),
                "wk": _wqk_layout(kp[rows, :][idx, :].T.astype(bf)),
                "wv": _wv_layout(vp[rows, :].T.astype(bf)),
                "wo": _wo_layout(op[:, rows].T.astype(bf)),
                "csf": csf,
                "dmask": dmask,
            }
        )

    res = run_bass_kernel_spmd(nc, in_maps, core_ids=list(range(NCORES)))
    LAST_RESULTS = res
    outp = np.zeros((B, S, D), dtype=np.float32)
    for c in range(NCORES):
        outp[c // 4] += res.results[c]["out"].astype(np.float32)
    return outp


# revision 52
# speedup vs baseline: 1.0023x; 1.0023x over previous
"""Multi-head attention with RoPE - Trainium2 Bass/Tile kernel.

Problem (hardcoded): B=2, S=2048, D=1024, H=16 heads, d_k=64, causal,
RoPE (theta=10000) on Q/K, fp32 reference.

Sharding: 8 cores = 2 batches x 4 head-groups (tensor parallel over heads,
data parallel over batch). Each core: QKV projections for its 4 heads,
RoPE, causal attention, and its o_proj row-slice -> partial [S, D] output.
Host gather sums the 4 partials per batch (the row-parallel all-reduce).

Fully-fused single-pipeline schedule (PE is the pacer at ~117us busy;
TimelineSim 144.7us vs the 210.8us phase-serial fp32 baseline):
  - All SBUF-resident tensors are bf16 (DMA halved, DVE 2x tensor-tensor);
    PSUM stays fp32.  Measured rel err ~5e-3 (gate is 2e-2).
  - One flex PSUM pool ([P,2,CH] x 2bufs = 4 banks) serves score pairs,
    projection psum groups, and o_proj psum; 4 more banks hold the
    per-head PV accumulators.  Projection/o_proj groups are interleaved
    into the attention k-tile loop as work-queue pops, so the pure-PE
    projection work fills every ACT-paced gap and qc-boundary bubble.
  - Scores for a head pair land in one [P,2,CH] tile -> ONE exp on ACT
    (bf16 out, causally restricted) -> PV accumulate per head.  PVs are
    software-pipelined one k-tile behind scores.  The causal diagonal is
    fixed by an in-place tril multiply on the exp'd stripe (DVE, off the
    critical path), keeping PV matmuls wide.
  - Softmax denominator comes from 64 ones-columns prepended per head in
    the V tile (psum partitions 0..63, base-0 for the DVE reciprocal);
    reciprocal_approx_fast + normalize folds into the attnT write
    (recips and normalize muls interleaved per head pair on DVE).
  - RoPE: Q/K psum is copied to bf16 (ACT), rotated on DVE via a
    stream_shuffle + 3 tensor-tensor ops against host-precomputed
    cos/sin tables (head dims pre-permuted so the rotation partner is a
    fixed 16-partition shuffle).
  - A burst of dummy matmuls on scratch warms the PE p-state under the
    initial DMA; outputs stream to DRAM per s-tile as o_proj completes.
"""

import numpy as np
import ml_dtypes

import concourse.tile as tile
from concourse import bacc, mybir
from concourse.bass_utils import run_bass_kernel_spmd

F32 = mybir.dt.float32
BF16 = mybir.dt.bfloat16
EXP = mybir.ActivationFunctionType.Exp
COPY = mybir.ActivationFunctionType.Copy

B, S, D, H, DK = 2, 2048, 1024, 16, 64
P = 128
NCORES = 8
HPC = 4  # heads per core
GD = HPC * DK  # 256 head dims per core
NDT = D // P  # 8 d-tiles
CH = 512  # q/s chunk
NQC = S // CH  # 4 chunks
KPC = CH // P  # 4 k-tiles per chunk
THETA = 10000.0
SCALE = 1.0 / 8.0  # 1/sqrt(DK)
SHUF_MASK = [(i + 16) % 32 for i in range(32)]

_CACHE = {}


def _build_nc():
    nc = bacc.Bacc("TRN2", target_bir_lowering=False, debug=False)
    # x arrives pre-transposed bf16: [D, S] (host layout prep)
    x = nc.dram_tensor("x", [D, S], BF16, kind="ExternalInput").ap()
    wq = nc.dram_tensor("wq", [P, 2, NDT, P], BF16, kind="ExternalInput").ap()
    wk = nc.dram_tensor("wk", [P, 2, NDT, P], BF16, kind="ExternalInput").ap()
    wv = nc.dram_tensor("wv", [P, NDT, GD], BF16, kind="ExternalInput").ap()
    wo = nc.dram_tensor("wo", [P, 2, D], BF16, kind="ExternalInput").ap()
    csf = nc.dram_tensor("csf", [P, 2, S], BF16, kind="ExternalInput").ap()
    dmask = nc.dram_tensor("dmask", [P, P], BF16, kind="ExternalInput").ap()
    out = nc.dram_tensor("out", [S, D], BF16, kind="ExternalOutput").ap()

    with tile.TileContext(nc) as tc:
        with (
            tc.tile_pool(name="const", bufs=1) as cpool,
            tc.tile_pool(name="wp", bufs=1) as wp,
            tc.tile_pool(name="csp", bufs=1) as csp,
            tc.tile_pool(name="xtp", bufs=4) as xtp,
            tc.tile_pool(name="rotp", bufs=1) as rotp,
            tc.tile_pool(name="vtp", bufs=1) as vtp,
            tc.tile_pool(name="attp", bufs=1) as attp,
            tc.tile_pool(name="qrp", bufs=4) as qrp,
            tc.tile_pool(name="shp", bufs=4) as shp,
            tc.tile_pool(name="t12", bufs=8) as t12,
            tc.tile_pool(name="ptp", bufs=20) as ptp,
            tc.tile_pool(name="rdp", bufs=6) as rdp,
            tc.tile_pool(name="osp", bufs=8) as osp,
            tc.tile_pool(name="ssp", bufs=2, space="PSUM") as ssp,
            tc.tile_pool(name="pvp", bufs=4, space="PSUM") as pvp,
        ):
            # ---- persistent SBUF tiles ----
            dmT = cpool.tile([P, P], BF16, tag="dm")
            scrT = cpool.tile([P, CH], BF16, tag="scr")
            woT = wp.tile([P, 2, D], BF16, tag="wo")
            wqT = wp.tile([P, 2, NDT, P], BF16, tag="wq")
            wkT = wp.tile([P, 2, NDT, P], BF16, tag="wk")
            wvT = wp.tile([P, NDT, GD], BF16, tag="wv")
            csC = [csp.tile([P, 2, CH], BF16, tag=f"cs{c}", name=f"cs{c}")
                   for c in range(NQC)]
            rotq = [[rotp.tile([P, CH], BF16, tag=f"rq{c}_{it}", name=f"rq{c}_{it}")
                     for it in range(2)] for c in range(NQC)]
            rotk = [[rotp.tile([P, CH], BF16, tag=f"rk{c}_{it}", name=f"rk{c}_{it}")
                     for it in range(2)] for c in range(NQC)]
            # per head: DK ones cols then DK value cols (softmax denominator
            # rides psum partitions 0..63 of the PV matmul).
            vt = [vtp.tile([P, KPC, HPC, 2 * DK], BF16, tag=f"vt{c}", name=f"vt{c}")
                  for c in range(NQC)]
            attn = [[attp.tile([P, CH], BF16, tag=f"at{c}_{i}", name=f"at{c}_{i}")
                     for i in range(2)] for c in range(NQC)]

            nc.gpsimd.memset(scrT[:], 0.0)

            # ---- input DMAs (SP queue), ordered by first use ----
            xrc = x.rearrange("(dt p) s -> p dt s", p=P)
            xt = []
            nc.sync.dma_start(wqT[:, 0], wq[:, 0])
            xt.append(xtp.tile([P, NDT, CH], BF16, tag="xt", name="xt0"))
            nc.sync.dma_start(xt[0][:, :, 0:CH // 2],
                              xrc[:, :, 0:CH // 2])
            nc.sync.dma_start(wqT[:, 1], wq[:, 1])
            nc.sync.dma_start(wkT[:, 0], wk[:, 0])
            nc.sync.dma_start(wkT[:, 1], wk[:, 1])
            nc.sync.dma_start(xt[0][:, :, CH // 2:CH],
                              xrc[:, :, CH // 2:CH])
            nc.sync.dma_start(wvT[:], wv)
            nc.sync.dma_start(csC[0][:], csf[:, :, 0:CH])
            nc.sync.dma_start(dmT[:], dmask)
            nc.sync.dma_start(woT[:], wo)
            for c in range(1, NQC):
                qsl = slice(c * CH, (c + 1) * CH)
                xt.append(xtp.tile([P, NDT, CH], BF16, tag="xt", name=f"xt{c}"))
                nc.sync.dma_start(xt[c][:], xrc[:, :, qsl])
                nc.sync.dma_start(csC[c][:], csf[:, :, qsl])

            # ones columns for the denominator (Pool, no deps)
            for c in range(NQC):
                for h in range(HPC):
                    nc.gpsimd.memset(vt[c][:, :, h, 0:DK], 1.0)

            # warm the PE p-state under the initial DMA: dummy matmuls on
            # scratch (results discarded; first real matmul then runs at
            # full clock)
            wps = ssp.tile([P, 2, CH], F32, tag="ss", name="warm")
            for _ in range(4):
                nc.tensor.matmul(wps[:, 0, :], scrT[0:P, 0:P], scrT[:],
                                 start=True, stop=True)

            # ---- work-queue pieces (popped into the kt loop) ----
            # half-width (256-moving) psum groups: each holds its flex
            # banks ~1.3us (856ns matmuls + ~400ns copy), matching the
            # exp/o_grp consumers so the 2-buffer rotation never stalls
            qraw_live = {}

            def qk_group(c, tgt, it, hw):
                """projection for one (q/k, it) half; RoPE after 2nd half"""
                wT, rot = (wqT, rotq[c]) if tgt == 0 else (wkT, rotk[c])
                HC = CH // 2
                hsl = slice(hw * HC, (hw + 1) * HC)
                ps = ssp.tile([P, 2, CH], F32, tag="ss",
                              name=f"qk{c}_{tgt}_{it}_{hw}")
                for dt in range(NDT):
                    nc.tensor.matmul(
                        ps[:, 0, 0:HC],
                        wT[:, it, dt, :],
                        xt[c][:, dt, hsl],
                        start=(dt == 0),
                        stop=(dt == NDT - 1),
                    )
                if hw == 0:
                    qraw_live[(c, tgt, it)] = qrp.tile(
                        [P, CH], BF16, tag="qraw",
                        name=f"qraw{c}_{tgt}_{it}")
                qraw = qraw_live[(c, tgt, it)]
                nc.scalar.activation(qraw[:, hsl], ps[:, 0, 0:HC], COPY)
                if hw == 1:
                    del qraw_live[(c, tgt, it)]
                    t1 = t12.tile([P, CH], BF16, tag="t1")
                    nc.vector.tensor_mul(t1[:], qraw[:], csC[c][:, 0, :])
                    sh = shp.tile([P, CH], BF16, tag="sh")
                    nc.vector.stream_shuffle(sh[:], qraw[:], SHUF_MASK)
                    t2 = t12.tile([P, CH], BF16, tag="t2")
                    nc.vector.tensor_mul(t2[:], sh[:], csC[c][:, 1, :])
                    nc.vector.tensor_add(rot[it][:], t1[:], t2[:])

            def v_group(c, stl):
                """V projection for one s-tile of chunk c"""
                ps = ssp.tile([P, 2, CH], F32, tag="ss", name=f"v{c}_{stl}")
                for dt in range(NDT):
                    nc.tensor.matmul(
                        ps[:, 0, 0:GD],
                        xt[c][:, dt, stl * P:(stl + 1) * P],
                        wvT[:, dt, :],
                        start=(dt == 0),
                        stop=(dt == NDT - 1),
                    )
                nc.scalar.activation(
                    vt[c][:, stl, :, DK:2 * DK],
                    ps[:, 0, 0:GD].rearrange("p (h d) -> p h d", h=HPC),
                    COPY,
                )

            def proj_chunk_work(c):
                w = []
                for tgt in range(2):
                    for it in range(2):
                        for hw in range(2):
                            w.append(lambda c=c, t=tgt, i=it, g=hw:
                                     qk_group(c, t, i, g))
                for stl in range(KPC):
                    w.append(lambda c=c, s=stl: v_group(c, s))
                return w

            def o_grp(qc, stl):
                """full-width o_proj for one s-tile (flex psum insertion)"""
                st = qc * KPC + stl
                psf = ssp.tile([P, 2, CH], F32, tag="ss", name=f"psf{st}")
                for jc in range(2):
                    jsl = slice(jc * CH, (jc + 1) * CH)
                    for itx in range(2):
                        # psum matmul dst must stay within one 2KB bank
                        nc.tensor.matmul(
                            psf[:, jc, :],
                            attn[qc][itx][:, stl * P:(stl + 1) * P],
                            woT[:, itx, jsl],
                            start=(itx == 0),
                            stop=(itx == 1),
                        )
                ost = osp.tile([P, D], BF16, tag="ost")
                # GPSIMD cannot read PSUM: one full-width DVE copy frees
                # the flex banks sooner than two serial half-copies
                nc.vector.tensor_copy(
                    ost[:], psf[:].rearrange('p a b -> p (a b)'))
                nc.sync.dma_start(out[st * P:(st + 1) * P, :], ost[:])

            # chunk-0 projections run up front, hw-major: the four
            # hw0 half-groups need only the first x column-half, so PE
            # rolls through all QK projections while x0b/wv stream in
            for hw in range(2):
                for tgt in range(2):
                    for it in range(2):
                        qk_group(0, tgt, it, hw)
            for stl in range(KPC):
                v_group(0, stl)

            work = []   # pending proj groups for the next chunk
            oq = []     # pending o_proj s-tiles
            # a few chunk-1 groups run before qc0 to fill the rope(0) wait
            head_start = proj_chunk_work(1)
            for w in head_start[:10]:
                w()
            prefetched = {1: 4

            # ---------------- fused attention pipeline ----------------
            for qc in range(NQC):
                nkt = (qc + 1) * KPC
                if qc < NQC - 1:
                    w_all = proj_chunk_work(qc + 1)
                    work.extend(w_all[prefetched.get(qc + 1, 0):])
                # spread proj pops across this qc's kts (all must land
                # before the qc boundary)
                pops_per_kt = max(1, len(work) // nkt)

                psos = [pvp.tile([P, CH], F32, tag="pso", name=f"pso{qc}_{h}")
                        for h in range(HPC)]
                # two-deep PV pipeline: flush two kts behind so boundary
                # norm waits never gate the score chain
                pend = []

                def flush_front():
                    if not pend:
                        return
                    for dst, ptt, hf, vs, st_, sp_, lhs in pend.pop(0):
                        nc.tensor.matmul(dst, lhs, ptt[:, hf, vs:CH],
                                         start=st_, stop=sp_)

                for kt in range(nkt):
                    kc, ktl = kt // KPC, kt % KPC
                    dj = kt - KPC * qc
                    vs = max(0, dj) * P
                    # score width: restrict when it stays >=256 wide
                    svs = vs if dj > 0 else 0  # bf16 matmul has no narrow-width penalty
                    pts = []
                    for it in range(2):
                        ss = ssp.tile([P, 2, CH], F32, tag="ss",
                                      name=f"ss{qc}_{kt}_{it}")
                        for hf in range(2):
                            prg = slice(hf * DK, (hf + 1) * DK)
                            nc.tensor.matmul(
                                ss[:, hf, svs:CH],
                                rotk[kc][it][prg, ktl * P:(ktl + 1) * P],
                                rotq[qc][it][prg, svs:CH],
                                start=True, stop=True,
                            )
                        pt = ptp.tile([P, 2, CH], BF16, tag="pt")
                        nc.scalar.activation(
                            pt[:, :, vs:CH], ss[:, :, vs:CH], EXP,
                            scale=SCALE)
                        if dj >= 0:
                            # fix the diagonal stripe of both heads in
                            # place; the consuming PV fires next kt, so
                            # this DVE multiply is off the critical path
                            nc.vector.tensor_mul(
                                pt[:, :, vs:vs + P],
                                pt[:, :, vs:vs + P],
                                dmT[:].unsqueeze(1).to_broadcast([P, 2, P]),
                            )
                        pts.append(pt)
                    if len(pend) >= 1:
                        flush_front()
                    pend.append([
                        (psos[h][:, vs:CH], pts[h // 2], h % 2, vs,
                         kt == 0, kt == nkt - 1, vt[kc][:, ktl, h, :])
                        for h in range(HPC)])
                    # pops at kt END: the flex buffer grabbed next waits on
                    # a fast copy, not an exp; the popped group absorbs the
                    # exp latency instead of the score chain
                    for _ in range(pops_per_kt):
                        if work:
                            work.pop(0)()
                    if qc == NQC - 1 and kt >= 1 and oq:
                        oq.pop(0)()
                while work:
                    work.pop(0)()
                while pend:
                    flush_front()
                rdens = []
                for h in range(HPC):
                    rden = rdp.tile([DK, CH], F32, tag="rden",
                                    name=f"rden{qc}_{h}")
                    nc.vector.reciprocal_approx_fast(
                        out=rden[:], in_=psos[h][0:DK, :])
                    rdens.append(rden)
                    if h % 2 == 1:
                        for hh in (h - 1, h):
                            dst = attn[qc][hh // 2][
                                (hh % 2) * DK:(hh % 2 + 1) * DK, :]
                            nc.vector.tensor_mul(
                                dst, psos[hh][DK:P, :], rdens[hh][:])
                if qc < NQC - 1:
                    for stl in range(KPC):
                        oq.append(lambda q=qc, s=stl: o_grp(q, s))
                else:
                    # tail: the last qc's o_proj drains through the freed
                    # PV banks (deep rotation, no flex-pool coupling)
                    while oq:
                        oq.pop(0)()
                    for stl in range(KPC):
                        st = qc * KPC + stl
                        ost = osp.tile([P, D], BF16, tag="ost",
                                       name=f"ostt{st}")
                        for jc in range(2):
                            jsl = slice(jc * CH, (jc + 1) * CH)
                            psf = pvp.tile([P, CH], F32, tag="pso",
                                           name=f"psft{st}_{jc}")
                            for itx in range(2):
                                nc.tensor.matmul(
                                    psf[:],
                                    attn[qc][itx][:, stl * P:(stl + 1) * P],
                                    woT[:, itx, jsl],
                                    start=(itx == 0),
                                    stop=(itx == 1),
                                )
                            if jc == 0:
                                nc.vector.tensor_copy(ost[:, jsl], psf[:])
                            else:
                                nc.scalar.activation(ost[:, jsl], psf[:], COPY)
                        nc.sync.dma_start(out[st * P:(st + 1) * P, :], ost[:])
            while oq:
                oq.pop(0)()
    nc.compile()
    return nc


def _tables():
    r = np.arange(P)
    j = 16 * ((r % 64) // 32) + (r % 16)
    inv = THETA ** (-(2.0 * j) / DK)
    ang = np.arange(S)[None, :] * inv[:, None]
    cosf = np.cos(ang)
    sgn = np.where((r % 32) < 16, -1.0, 1.0)
    sinf = np.sin(ang) * sgn[:, None]
    csf = np.ascontiguousarray(
        np.stack([cosf, sinf], axis=1)).astype(ml_dtypes.bfloat16)
    dmask = np.where(
        np.arange(P)[:, None] <= np.arange(P)[None, :], 1.0, 0.0
    ).astype(ml_dtypes.bfloat16)  # tril01: 1 where k <= q
    return csf, dmask


def _head_perm():
    # sbuf row r (within a head) <- original head dim perm[r]:
    # windows of 32 rows = [16 even dims, 16 odd dims]
    r = np.arange(DK)
    w = r // 32
    idx = r % 32
    return np.where(idx < 16, 32 * w + 2 * idx, 32 * w + 2 * (idx - 16) + 1)


def _wqk_layout(w):  # [D, GD] -> [P, 2(it), NDT, P]
    return np.ascontiguousarray(
        w.reshape(NDT, P, 2, P).transpose(1, 2, 0, 3))


def _wv_layout(w):  # [D, GD] -> [P, NDT, GD]
    return np.ascontiguousarray(w.reshape(NDT, P, GD).transpose(1, 0, 2))


def _wo_layout(w):  # [GD, D] -> [P, 2(it), D]
    return np.ascontiguousarray(w.reshape(2, P, D).transpose(1, 0, 2))


LAST_RESULTS = None


def kernel(**inputs):
    global LAST_RESULTS
    bf = ml_dtypes.bfloat16
    x = np.asarray(inputs["in_features"], dtype=np.float32)
    qp = np.asarray(inputs["q_proj"], dtype=np.float32)
    kp = np.asarray(inputs["k_proj"], dtype=np.float32)
    vp = np.asarray(inputs["v_proj"], dtype=np.float32)
    op = np.asarray(inputs["o_proj"], dtype=np.float32)

    if "nc" not in _CACHE:
        _CACHE["nc"] = _build_nc()
        _CACHE["tables"] = _tables()
    nc = _CACHE["nc"]
    csf, dmask = _CACHE["tables"]
    perm = _head_perm()
    idx = (np.arange(HPC)[:, None] * DK + perm[None, :]).reshape(-1)

    in_maps = []
    for c in range(NCORES):
        b, g = c // 4, c % 4
        rows = slice(HPC * g * DK, HPC * (g + 1) * DK)
        in_maps.append(
            {
                "x": np.ascontiguousarray(x[b].T.astype(bf)),
                "wq": _wqk_layout(qp[rows, :][idx, :].T.astype(bf)),
                "wk": _wqk_layout(kp[rows, :][idx, :].T.astype(bf)),
                "wv": _wv_layout(vp[rows, :].T.astype(bf)),
                "wo": _wo_layout(op[:, rows].T.astype(bf)),
                "csf": csf,
                "dmask": dmask,
            }
        )

    res = run_bass_kernel_spmd(nc, in_maps, core_ids=list(range(NCORES)))
    LAST_RESULTS = res
    outp = np.zeros((B, S, D), dtype=np.float32)
    for c in range(NCORES):
        outp[c // 4] += res.results[c]["out"].astype(np.float32)
    return outp
